# revision 5
# baseline (speedup 1.0000x reference)
"""Multi-head attention block on 8 Trainium2 NeuronCores.

Reference computation (per batch b of 4, N=2048, D=768, 12 heads x 64):
    qkv = x @ Wqkv; q,k,v = split(qkv)
    out = softmax(q @ k.T / 8) @ v   (per head)
    y   = concat_heads(out) @ Wout + bout

Sharding: 8 cores = 4 batches x 2 head-groups (6 heads each).  Each core
computes its batch's QKV projection for its 6 heads, full attention for
those heads, and a partial output projection (contracting only its heads'
rows of Wout).  The host sums the two head-group partials per batch and
adds the bias (the "all-reduce after to_out" done on host).

V3 design (ScalarE-dense pipeline).  The kernel-wide floor is the
softmax exp: 6 heads x 2048^2 elements = 25.2M / (128 lanes @ 1.2GHz)
= 164us of ScalarE busy time, plus ~0.3us per ACTIVATE.  Everything else
is scheduled around keeping ACT 100% busy:

  - all matmul tiles are bf16 (DMA'd as bf16 from host-converted
    inputs): halves DMA + SBUF, makes LDWEIGHTS cheap/FWL-able.  PSUM
    accumulation stays fp32; rel err ~1e-3, gate is 2e-2.
  - attention tick = (head-pair, 512-wide q block, k tile i): both
    heads' transposed scores in ONE psum tile sc[128, 2, 512] (2 banks)
    written by 2 row-tiled MMs; ONE 1024-wide exp (1147ns) -> pt bf16;
    attn@V one tick later (software pipeline) so PE never waits on ACT.
  - PSUM: "s" 2x2 banks (sc), "o" 2x1 (oacc, 65 rows: row 64 is the
    softmax denominator from the all-ones V column), "p" 2x1 dedicated
    to the projection GEMMs -- proj work never steals the exp chain's
    banks (the previous kernel's main stall).
  - x is DMA'd column-chunk-major so the first q/k chunks + v tiles are
    ready ~4us in; qkv/out projections are emitted between ticks (mids)
    sized ~1us so the 2-deep sc/pt buffers hide them.
"""

import os
import sys
import numpy as np

for _p in ("/opt/trn_rl_repo", "/opt/pypackages"):
    if os.path.isdir(_p) and _p not in sys.path:
        sys.path.append(_p)

import concourse.bass as bass
import concourse.mybir as mybir
import concourse.tile as tile
from concourse import bacc

# bf16 matmuls emit explicit InstLdweights, which walrus's ldw-opt pass
# (only relevant for self-loading f32r matmuls) crashes on -- keep the
# default --enable-ldw-opt=false.  Microbench showed f32r/bf16 weight
# loads are hidden by the PE reorder window anyway (same-stationary vs
# new-stationary-every-MM differ by ~12ns/MM).
LDW_OPT = False
import concourse.bass_utils as _bass_utils
if not getattr(_bass_utils, "_ldw_opt_patched", False):
    _orig_run_command = _bass_utils.run_command

    def _run_command_ldw(cmd, **kw):
        if LDW_OPT:
            cmd = ["--enable-ldw-opt=true" if c == "--enable-ldw-opt=false"
                   else c for c in cmd]
        return _orig_run_command(cmd, **kw)

    _bass_utils.run_command = _run_command_ldw
    _bass_utils._ldw_opt_patched = True

F32 = mybir.dt.float32
F32R = mybir.dt.float32r
BF16 = mybir.dt.bfloat16

P = 128          # partitions
N = 2048         # sequence length
D = 768          # model dim
HD = 64          # head dim
NHPC = 6         # heads per core
NPAIR = 3        # head pairs per core
KT = D // P      # 6 feature tiles
NT = N // P      # 16 sequence tiles
GCOLS = NHPC * HD          # 384 = this core's slice of inner dim
NQB = 4                    # n_q blocks
QB = N // NQB              # 512 block width
EXP = mybir.ActivationFunctionType.Exp
SCALE = 1.0 / np.sqrt(HD)


def build_nc(reps=1):
    nc = bacc.Bacc("TRN2", target_bir_lowering=False, debug=False,
                   num_devices=8)
    xT_d = nc.dram_tensor("xT", [D, N], BF16, kind="ExternalInput").ap()
    wq_d = nc.dram_tensor("wq", [D, GCOLS], BF16, kind="ExternalInput").ap()
    wk_d = nc.dram_tensor("wk", [D, GCOLS], BF16, kind="ExternalInput").ap()
    wv_d = nc.dram_tensor("wv", [D, GCOLS], BF16, kind="ExternalInput").ap()
    wo_d = nc.dram_tensor("wo", [GCOLS, D], BF16, kind="ExternalInput").ap()
    out_d = nc.dram_tensor("out", [N, D], F32, kind="ExternalOutput").ap()

    with tile.TileContext(nc) as tc, \
         nc.allow_low_precision(reason="bf16 matmuls"):
      for _rep in range(reps):
        with tc.tile_pool(name="persist", bufs=1) as pp, \
             tc.tile_pool(name="psS", bufs=2, space="PSUM") as psS, \
             tc.tile_pool(name="psO", bufs=2, space="PSUM") as psO, \
             tc.tile_pool(name="psP", bufs=2, space="PSUM") as psP, \
             tc.tile_pool(name="ptp", bufs=3) as ptp, \
             tc.tile_pool(name="obp", bufs=3) as obp, \
             tc.tile_pool(name="rp", bufs=2) as rp:
            ones = pp.tile([1, HD], F32R, tag="ones")
            nc.vector.memset(ones[:].bitcast(F32), 1.0)
            qT = pp.tile([P, NPAIR, N], BF16, tag="qT")
            kT = pp.tile([P, NPAIR, N], BF16, tag="kT")
            v = pp.tile([P, NT, NHPC, HD + 1], BF16, tag="v")
            oT = pp.tile([P, NPAIR, N], BF16, tag="oT")
            wo_sb = pp.tile([P, NPAIR, D], BF16, tag="wo")
            xt = pp.tile([P, KT, N], BF16, tag="xt")
            wv_sb = pp.tile([P, KT, GCOLS], BF16, tag="wv")
            wq_sb = pp.tile([P, NPAIR, KT, P], BF16, tag="wqs")
            wk_sb = pp.tile([P, NPAIR, KT, P], BF16, tag="wks")
            nc.vector.memset(v[:, :, :, HD:HD + 1], 1.0)

            # ---- input DMA schedule ----
            # Batched multi-dim transfers: one dma_start per logical
            # tensor chunk (the ~2us per-DMA completion latency was the
            # whole prologue cost when issued per 128-row block).

            xT_r = xT_d.rearrange("(kt p) n -> p kt n", p=P)
            wq_r = wq_d.rearrange("(kt p) g -> p kt g", p=P)
            wk_r = wk_d.rearrange("(kt p) g -> p kt g", p=P)
            wv_r = wv_d.rearrange("(kt p) g -> p kt g", p=P)
            wo_r = wo_d.rearrange("(hp p) d -> p hp d", p=P)
            # ACT HWDGE ring: head-pair-0 q/k weights (needed first),
            # then the rest of the q/k weights.
            nc.scalar.dma_start(wq_sb[:, 0, :, :], wq_r[:, :, 0:P])
            nc.scalar.dma_start(wk_sb[:, 0, :, :], wk_r[:, :, 0:P])
            # SP HWDGE ring: x, column-chunk-major so chunk c lands
            # ~2.5us after chunk c-1.
            for c in range(NQB):
                cols = slice(c * QB, (c + 1) * QB)
                nc.sync.dma_start(xt[:, :, cols], xT_r[:, :, cols])
            # SWDGE: wv (needed by v_proj(0) right away), then wo.
            nc.gpsimd.dma_start(wv_sb[:, :, :], wv_r[:, :, :])
            for hp in range(1, NPAIR):
                cols = slice(hp * P, (hp + 1) * P)
                nc.scalar.dma_start(wq_sb[:, hp, :, :], wq_r[:, :, cols])
                nc.scalar.dma_start(wk_sb[:, hp, :, :], wk_r[:, :, cols])
            nc.gpsimd.dma_start(wo_sb[:, :, :], wo_r[:, :, :])

            # ---- projection helpers (dedicated "p" psum tag) ----
            def qk_proj(hp, which, c):
                w_sb, dst = ((wq_sb, qT) if which == "q" else (wk_sb, kT))
                cols = slice(c * QB, (c + 1) * QB)
                ps = psP.tile([P, QB], F32, tag="p", name="ps")
                for kt in range(KT):
                    nc.tensor.matmul(ps[:], w_sb[:, hp, kt, :],
                                     xt[:, kt, cols],
                                     start=(kt == 0), stop=(kt == KT - 1))
                nc.vector.tensor_copy(dst[:, hp, cols], ps[:])

            def v_proj(nt):
                psv = psP.tile([P, QB], F32, tag="p", name="psv")
                for kt in range(KT):
                    nc.tensor.matmul(psv[:, 0:GCOLS],
                                     xt[:, kt, nt * P:(nt + 1) * P],
                                     wv_sb[:, kt, :],
                                     start=(kt == 0), stop=(kt == KT - 1))
                nc.vector.tensor_copy(
                    v[:, nt, :, 0:HD],
                    psv[:, 0:GCOLS].rearrange("p (h d) -> p h d", h=NHPC))

            out_q = [nc.sync, nc.gpsimd]

            def outproj(nt):
                ob = obp.tile([P, D], F32, tag="ob", name="ob")
                for h in range(2):
                    hs = slice(h * GCOLS, (h + 1) * GCOLS)
                    po = psP.tile([P, QB], F32, tag="p", name="po")
                    for hp in range(NPAIR):
                        nc.tensor.matmul(po[:, 0:GCOLS],
                                         oT[:, hp, nt * P:(nt + 1) * P],
                                         wo_sb[:, hp, hs],
                                         start=(hp == 0),
                                         stop=(hp == NPAIR - 1))
                    nc.vector.tensor_copy(ob[:, hs], po[:, 0:GCOLS])
                out_q[nt % 2].dma_start(out_d[nt * P:(nt + 1) * P, :],
                                        ob[:, 0:D])

            def do_unit(u):
                if u[0] == "v":
                    v_proj(u[1])
                elif u[0] == "qk":
                    qk_proj(u[1], u[2], u[3])
                else:
                    outproj(u[1])

            # ---- attention: one global tick stream over all blocks,
            # attn@V one tick behind scores/exp so PE never waits on
            # ACT, and block boundaries pipeline (block b's last av +
            # evacuation overlap block b+1's first scores/exp).
            def evacuate(hp, b2, oacc):
                # row HD of oacc is the softmax denominator
                nqs = slice(b2 * QB, (b2 + 1) * QB)
                for h2 in range(2):
                    oslc = oT[h2 * HD:(h2 + 1) * HD, hp, nqs]
                    nc.vector.tensor_copy(oslc, oacc[h2][0:HD, :])
                    r = rp.tile([1, QB], F32R, tag="r")
                    nc.vector.reciprocal(r[:], oacc[h2][HD:HD + 1, :])
                    bcp = oacc[h2][0:HD, :]
                    nc.tensor.matmul(bcp[:, :], ones[:, :], r[:, :],
                                     start=True, stop=True)
                    nc.vector.tensor_mul(oslc, oslc, bcp)

            def attn_stream(blocks, sched):
                n_gt = len(blocks) * NT
                prev_pt = None
                oacc = None
                prev_oacc = None
                for gt in range(n_gt + 1):
                    blk, i = divmod(gt, NT)
                    if gt < n_gt:
                        hp, b2 = blocks[blk]
                        if i == 0:
                            oacc = [psO.tile([HD + 1, QB], F32, tag="o",
                                             name=f"oacc{h2}")
                                    for h2 in range(2)]
                        nqs = slice(b2 * QB, (b2 + 1) * QB)
                        sc = psS.tile([P, 2, QB], F32, tag="s", name="sc")
                        kslc = slice(i * P, (i + 1) * P)
                        for h2, lo in ((0, 0), (1, HD)):
                            nc.tensor.matmul(
                                sc[:, h2, :],
                                kT[lo:lo + HD, hp, kslc],
                                qT[lo:lo + HD, hp, nqs],
                                start=True, stop=True,
                                tile_position=(lo, 0))
                        pt = ptp.tile([P, 2, QB], BF16, tag="pt", name="pt")
                        nc.scalar.activation(pt[:], sc[:], EXP, scale=SCALE)
                    else:
                        pt = None
                    if gt >= 1:
                        pblk, pi = divmod(gt - 1, NT)
                        php, pb2 = blocks[pblk]
                        av_acc = prev_oacc if pi == NT - 1 and i == 0 \
                            else oacc
                        # (i==0 crossing: oacc was just re-allocated; the
                        # previous block's accumulator is prev_oacc)
                        for h2 in range(2):
                            nc.tensor.matmul(
                                av_acc[h2][:],
                                v[:, pi, 2 * php + h2, :],
                                prev_pt[:, h2, :],
                                start=(pi == 0), stop=(pi == NT - 1))
                        if pi == NT - 1:
                            evacuate(php, pb2, av_acc)
                    prev_pt = pt
                    if i == NT - 1 or gt == n_gt:
                        prev_oacc = oacc
                    for u in sched.get(gt, ()):
                        do_unit(u)

            # ---- prologue compute ----
            qk_proj(0, "q", 0)
            qk_proj(0, "k", 0)
            for nt in range(3):
                v_proj(nt)

            # ---- static schedule (keyed by per-block tick i) ----
            SCHED = {
                (0, 0): {0: [("qk", 0, "k", 1), ("v", 3)],
                         1: [("v", 4)], 2: [("v", 5)], 3: [("v", 6)],
                         4: [("qk", 0, "k", 2)],
                         5: [("v", 7)], 6: [("v", 8)], 7: [("v", 9)],
                         8: [("qk", 0, "k", 3)],
                         9: [("v", 10)], 10: [("v", 11)], 11: [("v", 12)],
                         12: [("v", 13)], 13: [("v", 14)],
                         14: [("v", 15), ("qk", 0, "q", 1)]},
                (0, 1): {1: [("qk", 1, "k", 0)], 3: [("qk", 1, "k", 1)],
                         5: [("qk", 1, "k", 2)], 7: [("qk", 1, "k", 3)],
                         9: [("qk", 1, "q", 0)], 11: [("qk", 0, "q", 2)]},
                (0, 2): {1: [("qk", 0, "q", 3)], 5: [("qk", 1, "q", 1)]},
                (0, 3): {1: [("qk", 1, "q", 2)]},
                (1, 0): {1: [("qk", 2, "k", 0)], 3: [("qk", 2, "k", 1)],
                         5: [("qk", 2, "k", 2)], 7: [("qk", 2, "k", 3)],
                         9: [("qk", 2, "q", 0)], 11: [("qk", 1, "q", 3)]},
                (1, 1): {1: [("qk", 2, "q", 1)]},
                (1, 2): {1: [("qk", 2, "q", 2)]},
                (1, 3): {1: [("qk", 2, "q", 3)]},
                (2, 0): {},
                (2, 1): {1: [("out", 0)], 5: [("out", 1)],
                         9: [("out", 2)], 13: [("out", 3)]},
                (2, 2): {1: [("out", 4)], 5: [("out", 5)],
                         9: [("out", 6)], 13: [("out", 7)]},
                (2, 3): {1: [("out", 8)], 5: [("out", 9)],
                         9: [("out", 10)], 13: [("out", 11)]},
            }
            blocks = [(hp, b2) for hp in range(NPAIR) for b2 in range(NQB)]
            gsched = {}
            for bi, key in enumerate(blocks):
                for i, units in SCHED[key].items():
                    gsched[bi * NT + i] = units
            attn_stream(blocks, gsched)
            for nt in range(12, NT):
                outproj(nt)

    nc.compile()
    return nc


_NC_CACHE = None


def _get_nc():
    global _NC_CACHE
    if _NC_CACHE is None:
        _NC_CACHE = build_nc()
    return _NC_CACHE


def make_in_maps(x, Wqkv, Wout):
    import ml_dtypes
    bf16 = ml_dtypes.bfloat16
    in_maps = []
    for core in range(8):
        b, g = divmod(core, 2)
        cols = slice(g * GCOLS, (g + 1) * GCOLS)
        in_maps.append({
            "xT": np.ascontiguousarray(x[b].T).astype(bf16),
            "wq": np.ascontiguousarray(Wqkv[:, cols]).astype(bf16),
            "wk": np.ascontiguousarray(
                Wqkv[:, D + g * GCOLS:D + (g + 1) * GCOLS]).astype(bf16),
            "wv": np.ascontiguousarray(
                Wqkv[:, 2 * D + g * GCOLS:2 * D + (g + 1) * GCOLS]
            ).astype(bf16),
            "wo": np.ascontiguousarray(
                Wout[g * GCOLS:(g + 1) * GCOLS, :]).astype(bf16),
        })
    return in_maps


def assemble(results, bout):
    out = np.empty((4, N, D), np.float32)
    for b in range(4):
        out[b] = results[2 * b]["out"] + results[2 * b + 1]["out"] + bout[None, :]
    return out


def kernel(x, Wqkv, Wout, bout, _trace=False):
    from concourse.bass_utils import run_bass_kernel_spmd
    x = np.asarray(x, np.float32)
    Wqkv = np.asarray(Wqkv, np.float32)
    Wout = np.asarray(Wout, np.float32)
    bout = np.asarray(bout, np.float32)
    nc = _get_nc()
    res = run_bass_kernel_spmd(nc, make_in_maps(x, Wqkv, Wout),
                               list(range(8)), trace=_trace)
    out = assemble(res.results, bout)
    if _trace:
        return out, res
    return out


# revision 11
# speedup vs baseline: 1.0524x; 1.0524x over previous
"""Multi-head attention block on 8 Trainium2 NeuronCores.

Reference computation (per batch b of 4, N=2048, D=768, 12 heads x 64):
    qkv = x @ Wqkv; q,k,v = split(qkv)
    out = softmax(q @ k.T / 8) @ v   (per head)
    y   = concat_heads(out) @ Wout + bout

Sharding: 8 cores = 4 batches x 2 head-groups (6 heads each).  Each core
computes its batch's QKV projection for its 6 heads, full attention for
those heads, and a partial output projection (contracting only its heads'
rows of Wout).  The host sums the two head-group partials per batch and
adds the bias (the "all-reduce after to_out" done on host).

V3 design (ScalarE-dense pipeline).  The kernel-wide floor is the
softmax exp: 6 heads x 2048^2 elements = 25.2M / (128 lanes @ 1.2GHz)
= 164us of ScalarE busy time, plus ~0.3us per ACTIVATE.  Everything else
is scheduled around keeping ACT 100% busy:

  - all matmul tiles are bf16 (DMA'd as bf16 from host-converted
    inputs): halves DMA + SBUF, makes LDWEIGHTS cheap/FWL-able.  PSUM
    accumulation stays fp32; rel err ~1e-3, gate is 2e-2.
  - attention tick = (head-pair, 512-wide q block, k tile i): both
    heads' transposed scores in ONE psum tile sc[128, 2, 512] (2 banks)
    written by 2 row-tiled MMs; ONE 1024-wide exp (1147ns) -> pt bf16;
    attn@V one tick later (software pipeline) so PE never waits on ACT.
  - PSUM: "s" 2x2 banks (sc), "o" 2x1 (oacc, 65 rows: row 64 is the
    softmax denominator from the all-ones V column), "p" 2x1 dedicated
    to the projection GEMMs -- proj work never steals the exp chain's
    banks (the previous kernel's main stall).
  - x is DMA'd column-chunk-major so the first q/k chunks + v tiles are
    ready ~4us in; qkv/out projections are emitted between ticks (mids)
    sized ~1us so the 2-deep sc/pt buffers hide them.
"""

import os
import sys
import numpy as np

for _p in ("/opt/trn_rl_repo", "/opt/pypackages"):
    if os.path.isdir(_p) and _p not in sys.path:
        sys.path.append(_p)

import concourse.bass as bass
import concourse.mybir as mybir
import concourse.tile as tile
from concourse import bacc

# bf16 matmuls emit explicit InstLdweights, which walrus's ldw-opt pass
# (only relevant for self-loading f32r matmuls) crashes on -- keep the
# default --enable-ldw-opt=false.  Microbench showed f32r/bf16 weight
# loads are hidden by the PE reorder window anyway (same-stationary vs
# new-stationary-every-MM differ by ~12ns/MM).
LDW_OPT = False
import concourse.bass_utils as _bass_utils
if not getattr(_bass_utils, "_ldw_opt_patched", False):
    _orig_run_command = _bass_utils.run_command

    def _run_command_ldw(cmd, **kw):
        if LDW_OPT:
            cmd = ["--enable-ldw-opt=true" if c == "--enable-ldw-opt=false"
                   else c for c in cmd]
        return _orig_run_command(cmd, **kw)

    _bass_utils.run_command = _run_command_ldw
    _bass_utils._ldw_opt_patched = True

F32 = mybir.dt.float32
F32R = mybir.dt.float32r
BF16 = mybir.dt.bfloat16

P = 128          # partitions
N = 2048         # sequence length
D = 768          # model dim
HD = 64          # head dim
NHPC = 6         # heads per core
NPAIR = 3        # head pairs per core
KT = D // P      # 6 feature tiles
NT = N // P      # 16 sequence tiles
GCOLS = NHPC * HD          # 384 = this core's slice of inner dim
NQB = 4                    # n_q blocks
QB = N // NQB              # 512 block width
EXP = mybir.ActivationFunctionType.Exp
SCALE = 1.0 / np.sqrt(HD)


def build_nc(reps=1):
    nc = bacc.Bacc("TRN2", target_bir_lowering=False, debug=False,
                   num_devices=8)
    xT_d = nc.dram_tensor("xT", [D, N], BF16, kind="ExternalInput").ap()
    wq_d = nc.dram_tensor("wq", [D, GCOLS], BF16, kind="ExternalInput").ap()
    wk_d = nc.dram_tensor("wk", [D, GCOLS], BF16, kind="ExternalInput").ap()
    wv_d = nc.dram_tensor("wv", [D, GCOLS], BF16, kind="ExternalInput").ap()
    wo_d = nc.dram_tensor("wo", [GCOLS, D], BF16, kind="ExternalInput").ap()
    out_d = nc.dram_tensor("out", [N, D], F32, kind="ExternalOutput").ap()

    with tile.TileContext(nc) as tc, \
         nc.allow_low_precision(reason="bf16 matmuls"):
      for _rep in range(reps):
        with tc.tile_pool(name="persist", bufs=1) as pp, \
             tc.tile_pool(name="psS", bufs=2, space="PSUM") as psS, \
             tc.tile_pool(name="psO", bufs=2, space="PSUM") as psO, \
             tc.tile_pool(name="psP", bufs=2, space="PSUM") as psP, \
             tc.tile_pool(name="ptp", bufs=4) as ptp, \
             tc.tile_pool(name="obp", bufs=3) as obp, \
             tc.tile_pool(name="rp", bufs=2) as rp:
            ones = pp.tile([1, HD], F32R, tag="ones")
            nc.vector.memset(ones[:].bitcast(F32), 1.0)
            qT = pp.tile([P, NPAIR, N], BF16, tag="qT")
            kT = pp.tile([P, NPAIR, N], BF16, tag="kT")
            v = pp.tile([P, NT, NHPC, HD + 1], BF16, tag="v")
            oT = pp.tile([P, NPAIR, N], BF16, tag="oT")
            wo_sb = pp.tile([P, NPAIR, D], BF16, tag="wo")
            xt = pp.tile([P, KT, N], BF16, tag="xt")
            wv_sb = pp.tile([P, KT, GCOLS], BF16, tag="wv")
            wq_sb = pp.tile([P, NPAIR, KT, P], BF16, tag="wqs")
            wk_sb = pp.tile([P, NPAIR, KT, P], BF16, tag="wks")
            nc.vector.memset(v[:, :, :, HD:HD + 1], 1.0)

            # ---- input DMA schedule ----
            # Batched multi-dim transfers: one dma_start per logical
            # tensor chunk (the ~2us per-DMA completion latency was the
            # whole prologue cost when issued per 128-row block).

            xT_r = xT_d.rearrange("(kt p) n -> p kt n", p=P)
            wq_r = wq_d.rearrange("(kt p) g -> p kt g", p=P)
            wk_r = wk_d.rearrange("(kt p) g -> p kt g", p=P)
            wv_r = wv_d.rearrange("(kt p) g -> p kt g", p=P)
            wo_r = wo_d.rearrange("(hp p) d -> p hp d", p=P)
            # ACT HWDGE ring: head-pair-0 q/k weights (needed first),
            # then the rest of the q/k weights.
            nc.scalar.dma_start(wq_sb[:, 0, :, :], wq_r[:, :, 0:P])
            nc.scalar.dma_start(wk_sb[:, 0, :, :], wk_r[:, :, 0:P])
            # SP HWDGE ring: x, column-chunk-major so chunk c lands
            # ~2.5us after chunk c-1.
            for c in range(NQB):
                cols = slice(c * QB, (c + 1) * QB)
                nc.sync.dma_start(xt[:, :, cols], xT_r[:, :, cols])
            # SWDGE: wv (needed by v_proj(0) right away), then wo.
            nc.gpsimd.dma_start(wv_sb[:, :, :], wv_r[:, :, :])
            for hp in range(1, NPAIR):
                cols = slice(hp * P, (hp + 1) * P)
                nc.scalar.dma_start(wq_sb[:, hp, :, :], wq_r[:, :, cols])
                nc.scalar.dma_start(wk_sb[:, hp, :, :], wk_r[:, :, cols])
            nc.gpsimd.dma_start(wo_sb[:, :, :], wo_r[:, :, :])

            # ---- projection helpers (dedicated "p" psum tag) ----
            # Long accumulation chains starve the exp pipeline when a
            # whole 6-MM chain lands between two scores MMs, so each
            # unit is split into halves emitted on consecutive ticks.
            open_ps = {}
            open_ob = {}

            def qk_half(hp, which, c, half):
                w_sb, dst = ((wq_sb, qT) if which == "q" else (wk_sb, kT))
                cols = slice(c * QB, (c + 1) * QB)
                if half == 0:
                    ps = psP.tile([P, QB], F32, tag="p", name="ps")
                    open_ps[(which, hp, c)] = ps
                    kts = range(0, KT // 2)
                else:
                    ps = open_ps.pop((which, hp, c))
                    kts = range(KT // 2, KT)
                for kt in kts:
                    nc.tensor.matmul(ps[:], w_sb[:, hp, kt, :],
                                     xt[:, kt, cols],
                                     start=(kt == 0), stop=(kt == KT - 1))
                if half == 1:
                    nc.vector.tensor_copy(dst[:, hp, cols], ps[:])

            def v_half(nt, half):
                if half == 0:
                    psv = psP.tile([P, QB], F32, tag="p", name="psv")
                    open_ps[("v", nt)] = psv
                    kts = range(0, KT // 2)
                else:
                    psv = open_ps.pop(("v", nt))
                    kts = range(KT // 2, KT)
                for kt in kts:
                    nc.tensor.matmul(psv[:, 0:GCOLS],
                                     xt[:, kt, nt * P:(nt + 1) * P],
                                     wv_sb[:, kt, :],
                                     start=(kt == 0), stop=(kt == KT - 1))
                if half == 1:
                    nc.vector.tensor_copy(
                        v[:, nt, :, 0:HD],
                        psv[:, 0:GCOLS].rearrange("p (h d) -> p h d",
                                                  h=NHPC))

            out_q = [nc.sync, nc.gpsimd]

            def out_half(nt, h):
                if h == 0:
                    ob = obp.tile([P, D], F32, tag="ob", name="ob")
                    open_ob[nt] = ob
                else:
                    ob = open_ob.pop(nt)
                hs = slice(h * GCOLS, (h + 1) * GCOLS)
                po = psP.tile([P, QB], F32, tag="p", name="po")
                for hp in range(NPAIR):
                    nc.tensor.matmul(po[:, 0:GCOLS],
                                     oT[:, hp, nt * P:(nt + 1) * P],
                                     wo_sb[:, hp, hs],
                                     start=(hp == 0),
                                     stop=(hp == NPAIR - 1))
                nc.vector.tensor_copy(ob[:, hs], po[:, 0:GCOLS])
                if h == 1:
                    out_q[nt % 2].dma_start(out_d[nt * P:(nt + 1) * P, :],
                                            ob[:, 0:D])

            def qk_proj(hp, which, c):
                qk_half(hp, which, c, 0)
                qk_half(hp, which, c, 1)

            def v_proj(nt):
                v_half(nt, 0)
                v_half(nt, 1)

            def outproj(nt):
                out_half(nt, 0)
                out_half(nt, 1)

            def do_unit(u):
                kind = u[0]
                if kind == "v":
                    v_proj(u[1])
                elif kind == "v1":
                    v_half(u[1], 0)
                elif kind == "v2":
                    v_half(u[1], 1)
                elif kind == "qk":
                    qk_proj(u[1], u[2], u[3])
                elif kind == "qk1":
                    qk_half(u[1], u[2], u[3], 0)
                elif kind == "qk2":
                    qk_half(u[1], u[2], u[3], 1)
                elif kind == "out":
                    outproj(u[1])
                elif kind == "outA":
                    out_half(u[1], 0)
                elif kind == "outB":
                    out_half(u[1], 1)

            # ---- attention: one global tick stream over all blocks,
            # attn@V one tick behind scores/exp so PE never waits on
            # ACT, and block boundaries pipeline (block b's last av +
            # evacuation overlap block b+1's first scores/exp).
            def evacuate(hp, b2, oacc):
                # row HD of oacc is the softmax denominator.  bcp
                # borrows a "p" psum slot so oacc is released by the
                # single fused mul (one DVE op per head instead of
                # three -- the boundary DVE burst was backing up the
                # pt slots and starving ACT).
                nqs = slice(b2 * QB, (b2 + 1) * QB)
                for h2 in range(2):
                    oslc = oT[h2 * HD:(h2 + 1) * HD, hp, nqs]
                    nc.vector.tensor_copy(oslc, oacc[h2][0:HD, :])
                    r = rp.tile([1, QB], F32R, tag="r")
                    nc.vector.reciprocal(r[:], oacc[h2][HD:HD + 1, :])
                    bcp = psP.tile([P, QB], F32, tag="p", name="bcp")
                    nc.tensor.matmul(bcp[0:HD, :], ones[:, :], r[:, :],
                                     start=True, stop=True)
                    nc.vector.tensor_mul(oslc, oslc, bcp[0:HD, :])

            def attn_stream(blocks, sched):
                # software pipeline: scores(gt) | exp(gt-1) | av(gt-2),
                # so ACT's exp never waits on same-tick PE work and a
                # ~0.7us proj insertion cannot starve it.
                n_gt = len(blocks) * NT
                sc_q = {}
                pt_q = {}
                oacc = None
                for gt in range(n_gt + 2):
                    if gt < n_gt:
                        blk, i = divmod(gt, NT)
                        hp, b2 = blocks[blk]
                        nqs = slice(b2 * QB, (b2 + 1) * QB)
                        sc = psS.tile([P, 2, QB], F32, tag="s", name="sc")
                        kslc = slice(i * P, (i + 1) * P)
                        for h2, lo in ((0, 0), (1, HD)):
                            nc.tensor.matmul(
                                sc[:, h2, :],
                                kT[lo:lo + HD, hp, kslc],
                                qT[lo:lo + HD, hp, nqs],
                                start=True, stop=True,
                                tile_position=(lo, 0))
                        sc_q[gt] = sc
                    e = gt - 1
                    if 0 <= e < n_gt:
                        sc = sc_q.pop(e)
                        pt = ptp.tile([P, 2, QB], BF16, tag="pt", name="pt")
                        nc.scalar.activation(pt[:], sc[:], EXP, scale=SCALE)
                        pt_q[e] = pt
                    a = gt - 2
                    if 0 <= a < n_gt:
                        ablk, ai = divmod(a, NT)
                        ahp, ab2 = blocks[ablk]
                        if ai == 0:
                            oacc = [psO.tile([HD + 1, QB], F32, tag="o",
                                             name=f"oacc{h2}")
                                    for h2 in range(2)]
                        pt = pt_q.pop(a)
                        for h2 in range(2):
                            nc.tensor.matmul(
                                oacc[h2][:], v[:, ai, 2 * ahp + h2, :],
                                pt[:, h2, :],
                                start=(ai == 0), stop=(ai == NT - 1))
                        if ai == NT - 1:
                            evacuate(ahp, ab2, oacc)
                    for u in sched.get(gt, ()):
                        do_unit(u)

            # ---- prologue compute ----
            qk_proj(0, "q", 0)
            qk_proj(0, "k", 0)
            for nt in range(3):
                v_proj(nt)

            # ---- static schedule (keyed by per-block tick i) ----
            # Block (0,0) is PE/DMA-bound warmup (v deadlines), so it
            # keeps whole units; later blocks get half-units, one per
            # tick, and block-boundary ticks (0,1) stay clear.
            def qkh(hp, w, c, at):
                return {at: [("qk1", hp, w, c)], at + 1: [("qk2", hp, w, c)]}

            def outh(nt, at):
                return {at: [("outA", nt)], at + 1: [("outB", nt)]}

            def merge(*ds):
                out = {}
                for d in ds:
                    for k, units in d.items():
                        out.setdefault(k, []).extend(units)
                return out

            SCHED = {
                (0, 0): {0: [("qk", 0, "k", 1), ("v", 3)],
                         1: [("v", 4)], 2: [("v", 5)], 3: [("v", 6)],
                         4: [("qk", 0, "k", 2)],
                         5: [("v", 7)], 6: [("v", 8)], 7: [("v", 9)],
                         8: [("qk", 0, "k", 3)],
                         9: [("v", 10)], 10: [("v", 11)], 11: [("v", 12)],
                         12: [("v", 13)], 13: [("v", 14), ("qk1", 0, "q", 1)],
                         14: [("v", 15), ("qk2", 0, "q", 1)]},
                (0, 1): merge(qkh(1, "k", 0, 2), qkh(1, "k", 1, 4),
                              qkh(1, "k", 2, 6), qkh(1, "k", 3, 8),
                              qkh(1, "q", 0, 10), qkh(0, "q", 2, 12)),
                (0, 2): merge(qkh(0, "q", 3, 2), qkh(1, "q", 1, 6)),
                (0, 3): merge(qkh(1, "q", 2, 2)),
                (1, 0): merge(qkh(2, "k", 0, 2), qkh(2, "k", 1, 4),
                              qkh(2, "k", 2, 6), qkh(2, "k", 3, 8),
                              qkh(2, "q", 0, 10), qkh(1, "q", 3, 12)),
                (1, 1): merge(qkh(2, "q", 1, 2)),
                (1, 2): merge(qkh(2, "q", 2, 2)),
                (1, 3): merge(qkh(2, "q", 3, 2)),
                (2, 0): {},
                (2, 1): merge(outh(0, 2), outh(1, 6), outh(2, 10),
                              outh(3, 13)),
                (2, 2): merge(outh(4, 2), outh(5, 6), outh(6, 10),
                              outh(7, 13)),
                (2, 3): merge(outh(8, 2), outh(9, 6), outh(10, 10),
                              outh(11, 13)),
            }
            blocks = [(hp, b2) for hp in range(NPAIR) for b2 in range(NQB)]
            gsched = {}
            for bi, key in enumerate(blocks):
                for i, units in SCHED[key].items():
                    gsched[bi * NT + i] = units
            attn_stream(blocks, gsched)
            for nt in range(12, NT):
                outproj(nt)

    nc.compile()
    return nc


_NC_CACHE = None


def _get_nc():
    global _NC_CACHE
    if _NC_CACHE is None:
        _NC_CACHE = build_nc()
    return _NC_CACHE


def make_in_maps(x, Wqkv, Wout):
    import ml_dtypes
    bf16 = ml_dtypes.bfloat16
    in_maps = []
    for core in range(8):
        b, g = divmod(core, 2)
        cols = slice(g * GCOLS, (g + 1) * GCOLS)
        in_maps.append({
            "xT": np.ascontiguousarray(x[b].T).astype(bf16),
            "wq": np.ascontiguousarray(Wqkv[:, cols]).astype(bf16),
            "wk": np.ascontiguousarray(
                Wqkv[:, D + g * GCOLS:D + (g + 1) * GCOLS]).astype(bf16),
            "wv": np.ascontiguousarray(
                Wqkv[:, 2 * D + g * GCOLS:2 * D + (g + 1) * GCOLS]
            ).astype(bf16),
            "wo": np.ascontiguousarray(
                Wout[g * GCOLS:(g + 1) * GCOLS, :]).astype(bf16),
        })
    return in_maps


def assemble(results, bout):
    out = np.empty((4, N, D), np.float32)
    for b in range(4):
        out[b] = results[2 * b]["out"] + results[2 * b + 1]["out"] + bout[None, :]
    return out


def kernel(x, Wqkv, Wout, bout, _trace=False):
    from concourse.bass_utils import run_bass_kernel_spmd
    x = np.asarray(x, np.float32)
    Wqkv = np.asarray(Wqkv, np.float32)
    Wout = np.asarray(Wout, np.float32)
    bout = np.asarray(bout, np.float32)
    nc = _get_nc()
    res = run_bass_kernel_spmd(nc, make_in_maps(x, Wqkv, Wout),
                               list(range(8)), trace=_trace)
    out = assemble(res.results, bout)
    if _trace:
        return out, res
    return out


# revision 12
# speedup vs baseline: 1.0942x; 1.0396x over previous
"""Multi-head attention block on 8 Trainium2 NeuronCores.

Reference computation (per batch b of 4, N=2048, D=768, 12 heads x 64):
    qkv = x @ Wqkv; q,k,v = split(qkv)
    out = softmax(q @ k.T / 8) @ v   (per head)
    y   = concat_heads(out) @ Wout + bout

Sharding: 8 cores = 4 batches x 2 head-groups (6 heads each).  Each core
computes its batch's QKV projection for its 6 heads, full attention for
those heads, and a partial output projection (contracting only its heads'
rows of Wout).  The host sums the two head-group partials per batch and
adds the bias (the "all-reduce after to_out" done on host).

V3 design (ScalarE-dense pipeline).  The kernel-wide floor is the
softmax exp: 6 heads x 2048^2 elements = 25.2M / (128 lanes @ 1.2GHz)
= 164us of ScalarE busy time, plus ~0.3us per ACTIVATE.  Everything else
is scheduled around keeping ACT 100% busy:

  - all matmul tiles are bf16 (DMA'd as bf16 from host-converted
    inputs): halves DMA + SBUF, makes LDWEIGHTS cheap/FWL-able.  PSUM
    accumulation stays fp32; rel err ~1e-3, gate is 2e-2.
  - attention tick = (head-pair, 512-wide q block, k tile i): both
    heads' transposed scores in ONE psum tile sc[128, 2, 512] (2 banks)
    written by 2 row-tiled MMs; ONE 1024-wide exp (1147ns) -> pt bf16;
    attn@V one tick later (software pipeline) so PE never waits on ACT.
  - PSUM: "s" 2x2 banks (sc), "o" 2x1 (oacc, 65 rows: row 64 is the
    softmax denominator from the all-ones V column), "p" 2x1 dedicated
    to the projection GEMMs -- proj work never steals the exp chain's
    banks (the previous kernel's main stall).
  - x is DMA'd column-chunk-major so the first q/k chunks + v tiles are
    ready ~4us in; qkv/out projections are emitted between ticks (mids)
    sized ~1us so the 2-deep sc/pt buffers hide them.
"""

import os
import sys
import numpy as np

for _p in ("/opt/trn_rl_repo", "/opt/pypackages"):
    if os.path.isdir(_p) and _p not in sys.path:
        sys.path.append(_p)

import concourse.bass as bass
import concourse.mybir as mybir
import concourse.tile as tile
from concourse import bacc

# bf16 matmuls emit explicit InstLdweights, which walrus's ldw-opt pass
# (only relevant for self-loading f32r matmuls) crashes on -- keep the
# default --enable-ldw-opt=false.  Microbench showed f32r/bf16 weight
# loads are hidden by the PE reorder window anyway (same-stationary vs
# new-stationary-every-MM differ by ~12ns/MM).
LDW_OPT = False
import concourse.bass_utils as _bass_utils
if not getattr(_bass_utils, "_ldw_opt_patched", False):
    _orig_run_command = _bass_utils.run_command

    def _run_command_ldw(cmd, **kw):
        if LDW_OPT:
            cmd = ["--enable-ldw-opt=true" if c == "--enable-ldw-opt=false"
                   else c for c in cmd]
        return _orig_run_command(cmd, **kw)

    _bass_utils.run_command = _run_command_ldw
    _bass_utils._ldw_opt_patched = True

F32 = mybir.dt.float32
F32R = mybir.dt.float32r
BF16 = mybir.dt.bfloat16

P = 128          # partitions
N = 2048         # sequence length
D = 768          # model dim
HD = 64          # head dim
NHPC = 6         # heads per core
NPAIR = 3        # head pairs per core
KT = D // P      # 6 feature tiles
NT = N // P      # 16 sequence tiles
GCOLS = NHPC * HD          # 384 = this core's slice of inner dim
NQB = 4                    # n_q blocks
QB = N // NQB              # 512 block width
EXP = mybir.ActivationFunctionType.Exp
SCALE = 1.0 / np.sqrt(HD)


def build_nc(reps=1):
    nc = bacc.Bacc("TRN2", target_bir_lowering=False, debug=False,
                   num_devices=8)
    xT_d = nc.dram_tensor("xT", [D, N], BF16, kind="ExternalInput").ap()
    wq_d = nc.dram_tensor("wq", [D, GCOLS], BF16, kind="ExternalInput").ap()
    wk_d = nc.dram_tensor("wk", [D, GCOLS], BF16, kind="ExternalInput").ap()
    wv_d = nc.dram_tensor("wv", [D, GCOLS], BF16, kind="ExternalInput").ap()
    wo_d = nc.dram_tensor("wo", [GCOLS, D], BF16, kind="ExternalInput").ap()
    out_d = nc.dram_tensor("out", [N, D], F32, kind="ExternalOutput").ap()

    with tile.TileContext(nc) as tc, \
         nc.allow_low_precision(reason="bf16 matmuls"):
      for _rep in range(reps):
        with tc.tile_pool(name="persist", bufs=1) as pp, \
             tc.tile_pool(name="psS", bufs=2, space="PSUM") as psS, \
             tc.tile_pool(name="psO", bufs=2, space="PSUM") as psO, \
             tc.tile_pool(name="psP", bufs=2, space="PSUM") as psP, \
             tc.tile_pool(name="ptp", bufs=4) as ptp, \
             tc.tile_pool(name="obp", bufs=3) as obp, \
             tc.tile_pool(name="rp", bufs=2) as rp:
            ones = pp.tile([1, HD], F32R, tag="ones")
            nc.vector.memset(ones[:].bitcast(F32), 1.0)
            qT = pp.tile([P, NPAIR, N], BF16, tag="qT")
            kT = pp.tile([P, NPAIR, N], BF16, tag="kT")
            v = pp.tile([P, NT, NHPC, HD + 1], BF16, tag="v")
            oT = pp.tile([P, NPAIR, N], BF16, tag="oT")
            wo_sb = pp.tile([P, NPAIR, D], BF16, tag="wo")
            xt = pp.tile([P, KT, N], BF16, tag="xt")
            wv_sb = pp.tile([P, KT, GCOLS], BF16, tag="wv")
            wq_sb = pp.tile([P, NPAIR, KT, P], BF16, tag="wqs")
            wk_sb = pp.tile([P, NPAIR, KT, P], BF16, tag="wks")
            nc.vector.memset(v[:, :, :, HD:HD + 1], 1.0)

            # ---- input DMA schedule ----
            # Batched multi-dim transfers: one dma_start per logical
            # tensor chunk (the ~2us per-DMA completion latency was the
            # whole prologue cost when issued per 128-row block).

            # ACT HWDGE ring: head-pair-0 q/k weights (needed first),
            # then the rest of the q/k weights.
            for kt in range(KT):
                rows = slice(kt * P, (kt + 1) * P)
                nc.scalar.dma_start(wq_sb[:, 0, kt, :], wq_d[rows, 0:P])
                nc.scalar.dma_start(wk_sb[:, 0, kt, :], wk_d[rows, 0:P])
            # SP HWDGE ring: x, column-chunk-major so chunk c lands
            # ~2.5us after chunk c-1.
            for c in range(NQB):
                cols = slice(c * QB, (c + 1) * QB)
                for kt in range(KT):
                    rows = slice(kt * P, (kt + 1) * P)
                    nc.sync.dma_start(xt[:, kt, cols], xT_d[rows, cols])
            # SWDGE: wv (needed by v_proj(0) right away), then wo.
            for kt in range(KT):
                rows = slice(kt * P, (kt + 1) * P)
                nc.gpsimd.dma_start(wv_sb[:, kt, :], wv_d[rows, :])
            for hp in range(1, NPAIR):
                cols = slice(hp * P, (hp + 1) * P)
                for kt in range(KT):
                    rows = slice(kt * P, (kt + 1) * P)
                    nc.scalar.dma_start(wq_sb[:, hp, kt, :],
                                        wq_d[rows, cols])
                    nc.scalar.dma_start(wk_sb[:, hp, kt, :],
                                        wk_d[rows, cols])
            for hp in range(NPAIR):
                nc.gpsimd.dma_start(wo_sb[:, hp, :],
                                    wo_d[hp * P:(hp + 1) * P, :])

            # ---- projection helpers (dedicated "p" psum tag) ----
            # Long accumulation chains starve the exp pipeline when a
            # whole 6-MM chain lands between two scores MMs, so each
            # unit is split into halves emitted on consecutive ticks.
            open_ps = {}
            open_ob = {}

            def qk_half(hp, which, c, half):
                w_sb, dst = ((wq_sb, qT) if which == "q" else (wk_sb, kT))
                cols = slice(c * QB, (c + 1) * QB)
                if half == 0:
                    ps = psP.tile([P, QB], F32, tag="p", name="ps")
                    open_ps[(which, hp, c)] = ps
                    kts = range(0, KT // 2)
                else:
                    ps = open_ps.pop((which, hp, c))
                    kts = range(KT // 2, KT)
                for kt in kts:
                    nc.tensor.matmul(ps[:], w_sb[:, hp, kt, :],
                                     xt[:, kt, cols],
                                     start=(kt == 0), stop=(kt == KT - 1))
                if half == 1:
                    nc.vector.tensor_copy(dst[:, hp, cols], ps[:])

            def v_half(nt, half):
                if half == 0:
                    psv = psP.tile([P, QB], F32, tag="p", name="psv")
                    open_ps[("v", nt)] = psv
                    kts = range(0, KT // 2)
                else:
                    psv = open_ps.pop(("v", nt))
                    kts = range(KT // 2, KT)
                for kt in kts:
                    nc.tensor.matmul(psv[:, 0:GCOLS],
                                     xt[:, kt, nt * P:(nt + 1) * P],
                                     wv_sb[:, kt, :],
                                     start=(kt == 0), stop=(kt == KT - 1))
                if half == 1:
                    nc.vector.tensor_copy(
                        v[:, nt, :, 0:HD],
                        psv[:, 0:GCOLS].rearrange("p (h d) -> p h d",
                                                  h=NHPC))

            out_q = [nc.sync, nc.gpsimd]

            def out_half(nt, h):
                if h == 0:
                    ob = obp.tile([P, D], F32, tag="ob", name="ob")
                    open_ob[nt] = ob
                else:
                    ob = open_ob.pop(nt)
                hs = slice(h * GCOLS, (h + 1) * GCOLS)
                po = psP.tile([P, QB], F32, tag="p", name="po")
                for hp in range(NPAIR):
                    nc.tensor.matmul(po[:, 0:GCOLS],
                                     oT[:, hp, nt * P:(nt + 1) * P],
                                     wo_sb[:, hp, hs],
                                     start=(hp == 0),
                                     stop=(hp == NPAIR - 1))
                nc.vector.tensor_copy(ob[:, hs], po[:, 0:GCOLS])
                if h == 1:
                    out_q[nt % 2].dma_start(out_d[nt * P:(nt + 1) * P, :],
                                            ob[:, 0:D])

            def qk_proj(hp, which, c):
                qk_half(hp, which, c, 0)
                qk_half(hp, which, c, 1)

            def v_proj(nt):
                v_half(nt, 0)
                v_half(nt, 1)

            def outproj(nt):
                out_half(nt, 0)
                out_half(nt, 1)

            def do_unit(u):
                kind = u[0]
                if kind == "v":
                    v_proj(u[1])
                elif kind == "v1":
                    v_half(u[1], 0)
                elif kind == "v2":
                    v_half(u[1], 1)
                elif kind == "qk":
                    qk_proj(u[1], u[2], u[3])
                elif kind == "qk1":
                    qk_half(u[1], u[2], u[3], 0)
                elif kind == "qk2":
                    qk_half(u[1], u[2], u[3], 1)
                elif kind == "out":
                    outproj(u[1])
                elif kind == "outA":
                    out_half(u[1], 0)
                elif kind == "outB":
                    out_half(u[1], 1)

            # ---- attention: one global tick stream over all blocks,
            # attn@V one tick behind scores/exp so PE never waits on
            # ACT, and block boundaries pipeline (block b's last av +
            # evacuation overlap block b+1's first scores/exp).
            def evacuate(hp, b2, oacc):
                # row HD of oacc is the softmax denominator.  bcp
                # borrows a "p" psum slot so oacc is released by the
                # single fused mul (one DVE op per head instead of
                # three -- the boundary DVE burst was backing up the
                # pt slots and starving ACT).
                nqs = slice(b2 * QB, (b2 + 1) * QB)
                for h2 in range(2):
                    oslc = oT[h2 * HD:(h2 + 1) * HD, hp, nqs]
                    nc.vector.tensor_copy(oslc, oacc[h2][0:HD, :])
                    r = rp.tile([1, QB], F32R, tag="r")
                    nc.vector.reciprocal(r[:], oacc[h2][HD:HD + 1, :])
                    bcp = psP.tile([P, QB], F32, tag="p", name="bcp")
                    nc.tensor.matmul(bcp[0:HD, :], ones[:, :], r[:, :],
                                     start=True, stop=True)
                    nc.vector.tensor_mul(oslc, oslc, bcp[0:HD, :])

            def attn_stream(blocks, sched):
                # software pipeline: scores(gt) | exp(gt-1) | av(gt-2),
                # so ACT's exp never waits on same-tick PE work and a
                # ~0.7us proj insertion cannot starve it.
                n_gt = len(blocks) * NT
                sc_q = {}
                pt_q = {}
                oacc = None
                for gt in range(n_gt + 2):
                    if gt < n_gt:
                        blk, i = divmod(gt, NT)
                        hp, b2 = blocks[blk]
                        nqs = slice(b2 * QB, (b2 + 1) * QB)
                        sc = psS.tile([P, 2, QB], F32, tag="s", name="sc")
                        kslc = slice(i * P, (i + 1) * P)
                        for h2, lo in ((0, 0), (1, HD)):
                            nc.tensor.matmul(
                                sc[:, h2, :],
                                kT[lo:lo + HD, hp, kslc],
                                qT[lo:lo + HD, hp, nqs],
                                start=True, stop=True,
                                tile_position=(lo, 0))
                        sc_q[gt] = sc
                    e = gt - 1
                    if 0 <= e < n_gt:
                        sc = sc_q.pop(e)
                        pt = ptp.tile([P, 2, QB], BF16, tag="pt", name="pt")
                        nc.scalar.activation(pt[:], sc[:], EXP, scale=SCALE)
                        pt_q[e] = pt
                    a = gt - 2
                    if 0 <= a < n_gt:
                        ablk, ai = divmod(a, NT)
                        ahp, ab2 = blocks[ablk]
                        if ai == 0:
                            oacc = [psO.tile([HD + 1, QB], F32, tag="o",
                                             name=f"oacc{h2}")
                                    for h2 in range(2)]
                        pt = pt_q.pop(a)
                        for h2 in range(2):
                            nc.tensor.matmul(
                                oacc[h2][:], v[:, ai, 2 * ahp + h2, :],
                                pt[:, h2, :],
                                start=(ai == 0), stop=(ai == NT - 1))
                        if ai == NT - 1:
                            evacuate(ahp, ab2, oacc)
                    for u in sched.get(gt, ()):
                        do_unit(u)

            # ---- prologue compute ----
            qk_proj(0, "q", 0)
            qk_proj(0, "k", 0)
            for nt in range(3):
                v_proj(nt)

            # ---- static schedule (keyed by per-block tick i) ----
            # Block (0,0) is PE/DMA-bound warmup (v deadlines), so it
            # keeps whole units; later blocks get half-units, one per
            # tick, and block-boundary ticks (0,1) stay clear.
            def qkh(hp, w, c, at):
                return {at: [("qk1", hp, w, c)], at + 1: [("qk2", hp, w, c)]}

            def outh(nt, at):
                return {at: [("outA", nt)], at + 1: [("outB", nt)]}

            def merge(*ds):
                out = {}
                for d in ds:
                    for k, units in d.items():
                        out.setdefault(k, []).extend(units)
                return out

            SCHED = {
                (0, 0): {0: [("qk", 0, "k", 1), ("v", 3)],
                         1: [("v", 4)], 2: [("v", 5)], 3: [("v", 6)],
                         4: [("qk", 0, "k", 2)],
                         5: [("v", 7)], 6: [("v", 8)], 7: [("v", 9)],
                         8: [("qk", 0, "k", 3)],
                         9: [("v", 10)], 10: [("v", 11)], 11: [("v", 12)],
                         12: [("v", 13)], 13: [("v", 14), ("qk1", 0, "q", 1)],
                         14: [("v", 15), ("qk2", 0, "q", 1)]},
                (0, 1): merge(qkh(1, "k", 0, 2), qkh(1, "k", 1, 4),
                              qkh(1, "k", 2, 6), qkh(1, "k", 3, 8),
                              qkh(1, "q", 0, 10), qkh(0, "q", 2, 12)),
                (0, 2): merge(qkh(0, "q", 3, 2), qkh(1, "q", 1, 6)),
                (0, 3): merge(qkh(1, "q", 2, 2)),
                (1, 0): merge(qkh(2, "k", 0, 2), qkh(2, "k", 1, 4),
                              qkh(2, "k", 2, 6), qkh(2, "k", 3, 8),
                              qkh(2, "q", 0, 10), qkh(1, "q", 3, 12)),
                (1, 1): merge(qkh(2, "q", 1, 2)),
                (1, 2): merge(qkh(2, "q", 2, 2)),
                (1, 3): merge(qkh(2, "q", 3, 2)),
                (2, 0): {},
                (2, 1): merge(outh(0, 2), outh(1, 6), outh(2, 10),
                              outh(3, 13)),
                (2, 2): merge(outh(4, 2), outh(5, 6), outh(6, 10),
                              outh(7, 13)),
                (2, 3): merge(outh(8, 2), outh(9, 6), outh(10, 10),
                              outh(11, 13)),
            }
            blocks = [(hp, b2) for hp in range(NPAIR) for b2 in range(NQB)]
            gsched = {}
            for bi, key in enumerate(blocks):
                for i, units in SCHED[key].items():
                    gsched[bi * NT + i] = units
            attn_stream(blocks, gsched)
            for nt in range(12, NT):
                outproj(nt)

    nc.compile()
    return nc


_NC_CACHE = None


def _get_nc():
    global _NC_CACHE
    if _NC_CACHE is None:
        _NC_CACHE = build_nc()
    return _NC_CACHE


def make_in_maps(x, Wqkv, Wout):
    import ml_dtypes
    bf16 = ml_dtypes.bfloat16
    in_maps = []
    for core in range(8):
        b, g = divmod(core, 2)
        cols = slice(g * GCOLS, (g + 1) * GCOLS)
        in_maps.append({
            "xT": np.ascontiguousarray(x[b].T).astype(bf16),
            "wq": np.ascontiguousarray(Wqkv[:, cols]).astype(bf16),
            "wk": np.ascontiguousarray(
                Wqkv[:, D + g * GCOLS:D + (g + 1) * GCOLS]).astype(bf16),
            "wv": np.ascontiguousarray(
                Wqkv[:, 2 * D + g * GCOLS:2 * D + (g + 1) * GCOLS]
            ).astype(bf16),
            "wo": np.ascontiguousarray(
                Wout[g * GCOLS:(g + 1) * GCOLS, :]).astype(bf16),
        })
    return in_maps


def assemble(results, bout):
    out = np.empty((4, N, D), np.float32)
    for b in range(4):
        out[b] = results[2 * b]["out"] + results[2 * b + 1]["out"] + bout[None, :]
    return out


def kernel(x, Wqkv, Wout, bout, _trace=False):
    from concourse.bass_utils import run_bass_kernel_spmd
    x = np.asarray(x, np.float32)
    Wqkv = np.asarray(Wqkv, np.float32)
    Wout = np.asarray(Wout, np.float32)
    bout = np.asarray(bout, np.float32)
    nc = _get_nc()
    res = run_bass_kernel_spmd(nc, make_in_maps(x, Wqkv, Wout),
                               list(range(8)), trace=_trace)
    out = assemble(res.results, bout)
    if _trace:
        return out, res
    return out
